# revision 29
# baseline (speedup 1.0000x reference)
"""Trainium2 Bass kernel for nn_BinaryTokenClassificationModel (segment_reduce).

Math: logits[b,i,j] = dot(segmean(1+i), w_src) + dot(segmean(513+j), w_tgt) + b,
where segmean(s) is the mean of outputs[b] over the s-th consecutive run of
equal word_ids.  dot commutes with the segment mean, so per-token projections
proj[t] = x[t]·w_c suffice.  Design notes (from HW traces):

- HBM-DMA-bound: only tokens with segment id <= 1024 matter (~10.5MB/core);
  the 16 per-core DMA engines cap at ~360 GB/s => ~29us floor.  The joint
  SBUF bandwidth (DMA write + DVE reads/writes + ACT read, ~20B per element)
  is the second roofline, so exactly ONE wide multiply stream runs: DVE
  multiplies (1.22us/tile), the scalar engine's fused activation-accumulate
  reduces (1.07us), per 1.43us DMA tile slot.  Pool's f32 tensor_tensor is
  ~4x slow and poisons DVE when co-run; tensor_tensor_reduce and Pool's
  scalar_tensor_tensor compile but die on real hardware; DVE->PSUM writes
  are slower than SBUF writes.
- consts ride at the head of the x DMA queue (FIFO per queue) so tile-0
  compute is never starved; tail-only selector consts are slotted behind the
  fifth x pair.
- Ragged segment-sums accumulate on the PE into a pre-zeroed PSUM tile with
  accumulate-only (start=False) [128,1]-rhs matmuls, emitted inline per tile
  (start=True flags would corrupt other open accumulations; with none, tiny
  matmuls interleave freely).  lhsT = on-chip-generated s_lo one-hots; the
  second (big) one-hot batch is generated after pair 1 so it never delays the
  first multiply.  1/count folds into host constants (word_ids metadata).
- The tail streams: each region's mean (one tensor_scalar from PSUM) and its
  tgt broadcast-staircase matmul emit as soon as the region's last tile is
  in; only the last chunk's chain + 4 broadcast-adds + stores trail the loop.

Sharding: pure data parallel, one example (B=8) per NeuronCore (8 cores).
"""
import sys

for _p in ("/opt/trn_rl_repo", "/root/.axon_site/_ro/trn_rl_repo"):
    if _p not in sys.path:
        sys.path.append(_p)

from contextlib import ExitStack

import numpy as np

import concourse.bacc as bacc
import concourse.bass as bass
import concourse.tile as tile
from concourse import mybir
from concourse.bass_utils import run_bass_kernel_spmd

F32 = mybir.dt.float32
P = 128
H = 1024
AL = mybir.AluOpType

# pool column regions: (seg_chunk u, c) with c: 0=src (segs 1..512), 1=tgt
# (segs 513..1024).  seg s -> chunk u = s//128, slo = s%128.
REGIONS = [(0, 0), (1, 0), (2, 0), (3, 0), (4, 0),
           (4, 1), (5, 1), (6, 1), (7, 1), (8, 1)]
NREG = len(REGIONS)


def _build_nc(NT: int, plan: dict) -> bass.Bass:
    NCOL = plan["ncol"]
    KSPLIT = plan["ksplit"]          # cl columns needed by the first two pairs
    CL_CHUNKS = plan["cl_chunks"]    # [(emit_after_tile, k0, k1)]
    passes = plan["passes"]          # per half-tile: list of c values
    mms = plan["mms"]                # per half-tile: list of (q, k, c, stop)
    emit_after = plan["emit_after"]  # per half-tile: regions closing at i
    src_done_tile = plan["src_done_tile"]

    nc = bacc.Bacc("TRN2", target_bir_lowering=False, debug=False, num_devices=8)
    NCE = P + NCOL + NREG + 1        # early consts: iota | slo | rec | bias
    x_d = nc.declare_dram_parameter("x", [NT * P, H], F32, isOutput=False)
    ce_d = nc.declare_dram_parameter("consts", [P, NCE], F32, isOutput=False)
    ct_d = nc.declare_dram_parameter("consts_tail", [P, 3 * P], F32, isOutput=False)
    w_d = nc.declare_dram_parameter("wrow", [1, 2 * H], F32, isOutput=False)
    y_d = nc.declare_dram_parameter("y", [512, 512], F32, isOutput=True)

    with tile.TileContext(nc) as tc, ExitStack() as ctx:
        consts = ctx.enter_context(tc.tile_pool(name="consts", bufs=1))
        clp = ctx.enter_context(tc.tile_pool(name="clp", bufs=1))
        xpool = ctx.enter_context(tc.tile_pool(name="xp", bufs=9))
        scrv = ctx.enter_context(tc.tile_pool(name="scrv", bufs=3))
        vpool = ctx.enter_context(tc.tile_pool(name="vp", bufs=4))
        segp = ctx.enter_context(tc.tile_pool(name="segp", bufs=1))
        opool = ctx.enter_context(tc.tile_pool(name="op", bufs=4))
        pw_pool = ctx.enter_context(tc.tile_pool(name="pw", bufs=4, space="PSUM"))
        ppool_acc = ctx.enter_context(tc.tile_pool(name="pacc", bufs=1, space="PSUM"))
        ppool_sm = ctx.enter_context(tc.tile_pool(name="psm", bufs=2, space="PSUM"))

        # ---- head of the sync DMA stream: w row (gates the PE broadcast),
        # early consts, then x pairs; FIFO per queue means nothing starves.
        wrow = consts.tile([1, 2 * H], F32)
        nc.sync.dma_start(out=wrow, in_=w_d[:])
        cc = consts.tile([P, NCE], F32)
        nc.sync.dma_start(out=cc, in_=ce_d[:])

        iota = cc[:, 0:P]
        slo_mat = cc[:, P:P + NCOL]
        rec = cc[:, P + NCOL:P + NCOL + NREG]
        biascol = cc[:, NCE - 1:NCE]
        ct = consts.tile([P, 3 * P], F32)
        ident = ct[:, 0:P]
        s1 = ct[:, P:2 * P]
        s2 = ct[:, 2 * P:3 * P]

        # ---- broadcast w across partitions: ones-row stationary matmuls into
        # PSUM, then scalar-engine copies into SBUF (idle engines at start).
        ones_row = consts.tile([1, P], F32)
        nc.gpsimd.memset(ones_row, 1.0)
        wrep = consts.tile([P, 2 * H], F32)
        for q in range(4):
            pw = pw_pool.tile([P, 512], F32, tag="pw", name=f"pw{q}")
            nc.tensor.matmul(pw, lhsT=ones_row, rhs=wrow[:, 512 * q:512 * (q + 1)],
                             start=True, stop=True)
            nc.scalar.copy(out=wrep[:, 512 * q:512 * (q + 1)], in_=pw)

        # ---- s_lo one-hots (is_equal is DVE-only): only the first two pairs'
        # columns up front; the rest is generated after pair 1's multiplies.
        cl_all = clp.tile([P, NCOL, P], F32)
        nc.vector.tensor_tensor(
            out=cl_all[:, 0:KSPLIT],
            in0=iota.unsqueeze(1).to_broadcast((P, KSPLIT, P)),
            in1=slo_mat[:, 0:KSPLIT].unsqueeze(2).to_broadcast((P, KSPLIT, P)),
            op=AL.is_equal)

        pool_ps = ppool_acc.tile([P, NREG], F32)
        nc.vector.memset(pool_ps, 0.0)

        mean = segp.tile([P, NREG], F32)
        msrc_ps = ppool_sm.tile([P, 4], F32, tag="sm")
        msrc = segp.tile([P, 4], F32)
        rowb_ps = ppool_sm.tile([P, 512], F32, tag="sm")
        # tgt region q -> rowb staircase block (col range, ident slice)
        rowb_blk = {5: (0, 127, (1, 128)), 6: (127, 255, (0, 128)),
                    7: (255, 383, (0, 128)), 8: (383, 511, (0, 128)),
                    9: (511, 512, (0, 1))}

        # ---- main loop ----
        for g in range(NT // 2):
            x_pair = xpool.tile([P, 2, H], F32, name="x_pair", tag="x_pair")
            src = x_d[256 * g:256 * (g + 1), :].rearrange("(two p) h -> p two h", p=P)
            nc.sync.dma_start(out=x_pair, in_=src)
            if g == min(4, NT // 2 - 1):
                # tail-only selector consts: behind five pairs of x (keeps the
                # head tight), well before the first staircase needs them
                nc.sync.dma_start(out=ct, in_=ct_d[:])
            for half in range(2):
                i = 2 * g + half
                x_sub = x_pair[:, half, :]
                vs = {}
                for c in passes[i]:
                    v = vpool.tile([P, 1], F32, name="v", tag="v")
                    vs[c] = v
                    scr = scrv.tile([P, H], F32, name="scrv")
                    nc.vector.tensor_tensor(
                        out=scr, in0=x_sub, in1=wrep[:, c * H:(c + 1) * H],
                        op=AL.mult)
                    nc.scalar.activation(
                        out=scr, in_=scr,
                        func=mybir.ActivationFunctionType.Copy,
                        accum_out=v)
                for (q, k, c, stop) in mms[i]:
                    nc.tensor.matmul(pool_ps[:, q:q + 1], lhsT=cl_all[:, k, :],
                                     rhs=vs[c], start=False, stop=stop,
                                     skip_group_check=True)
                for q in emit_after[i]:
                    nc.vector.tensor_scalar(out=mean[:, q:q + 1],
                                            in0=pool_ps[:, q:q + 1],
                                            scalar1=rec[:, q:q + 1],
                                            scalar2=None, op0=AL.mult)
                    if q >= 5:
                        lo, hi, (a0, a1) = rowb_blk[q]
                        nc.tensor.matmul(rowb_ps[:, lo:hi],
                                         lhsT=mean[:, q:q + 1].to_broadcast((P, P)),
                                         rhs=ident[:, a0:a1], start=True, stop=True,
                                         skip_group_check=True)
                if i == src_done_tile:
                    # segs 1..512: s1/s2 selector matmuls + bias, mid-loop
                    nc.tensor.matmul(msrc_ps, lhsT=s1, rhs=mean[:, 0:4],
                                     start=True, stop=False, skip_group_check=True)
                    nc.tensor.matmul(msrc_ps, lhsT=s2, rhs=mean[:, 1:5],
                                     start=False, stop=True, skip_group_check=True)
                    nc.vector.tensor_scalar(out=msrc, in0=msrc_ps, scalar1=biascol,
                                            scalar2=None, op0=AL.add)
                # remaining one-hot chunks, emitted off the critical path in
                # pieces small enough not to lag the multiply stream
                for (at_i, k0, k1) in CL_CHUNKS:
                    if i == at_i and k1 > k0:
                        nc.vector.tensor_tensor(
                            out=cl_all[:, k0:k1],
                            in0=iota.unsqueeze(1).to_broadcast((P, k1 - k0, P)),
                            in1=slo_mat[:, k0:k1].unsqueeze(2).to_broadcast(
                                (P, k1 - k0, P)),
                            op=AL.is_equal)

        # ---- tail: 4 broadcast-adds + stores ----
        for k in range(4):
            lg = opool.tile([P, 512], F32, name="lg", tag="lg")
            if k % 2 == 0:
                nc.scalar.activation(out=lg, in_=rowb_ps,
                                     func=mybir.ActivationFunctionType.Identity,
                                     bias=msrc[:, k:k + 1], scale=1.0)
            else:
                nc.vector.tensor_scalar(out=lg, in0=rowb_ps, scalar1=msrc[:, k:k + 1],
                                        scalar2=None, op0=AL.add)
            nc.sync.dma_start(out=y_d[P * k:P * (k + 1), :], in_=lg)

    nc.compile()
    return nc


def _host_prep(inputs):
    x = np.ascontiguousarray(np.asarray(inputs["outputs"], dtype=np.float32))
    wid = np.asarray(inputs["word_ids"]).astype(np.int64)
    cw = np.asarray(inputs["classifier_w"], dtype=np.float32)
    bias = np.float32(np.asarray(inputs["classifier_b"]))
    B, L, Hd = x.shape
    assert (Hd, L) == (H, 4096) and B == 8
    assert int(inputs["num_src"]) == 512 and int(inputs["num_tgt"]) == 512

    # consecutive-run segment ids (attention_mask is all ones for this problem)
    new_seg = np.ones((B, L), np.int64)
    new_seg[:, 1:] = wid[:, 1:] != wid[:, :-1]
    seg = np.cumsum(new_seg, axis=1) - 1

    # token cutoff: segments beyond 1024 never reach the output
    cutoff = max(int(np.nonzero(seg[b] <= 1024)[0][-1]) for b in range(B))
    NPAIR = min((cutoff + 1 + 255) // 256, L // 256)
    NT = 2 * NPAIR
    Ltok = NT * P

    # per half-tile (128 contiguous tokens) region/column plan, union over cores
    segt = seg[:, :Ltok].reshape(B, NT, P)        # [B, NT, 128]
    valid = segt <= 1024
    u_of = segt // P
    c_of = (segt > 512).astype(np.int64)          # 0=src, 1=tgt
    qidx = {r: q for q, r in enumerate(REGIONS)}

    cols = []                        # (i, q) -> column index k
    mms = []                         # per i: [(q, k, c, stop)] (stop set below)
    passes = []                      # per i: list of c values
    last_touch = {}                  # q -> last tile index
    for i in range(NT):
        regs = set()
        for b in range(B):
            vb = valid[b, i]
            if not vb.any():
                continue
            for u, c in zip(u_of[b, i][vb], c_of[b, i][vb]):
                regs.add((int(u), int(c)))
        regs = sorted(regs, key=lambda r: qidx[r])
        tile_mms = []
        for r in regs:
            q = qidx[r]
            k = len(cols)
            cols.append((i, q))
            tile_mms.append([q, k, r[1], False])
            last_touch[q] = i
        mms.append(tile_mms)
        passes.append(sorted({r[1] for r in regs}))

    emit_after = [[] for _ in range(NT)]
    for q, i in last_touch.items():
        emit_after[i].append(q)
        for m in mms[i]:
            if m[0] == q:
                m[3] = True
    for e in emit_after:
        e.sort()
    NCOL = len(cols)
    src_done_tile = max((i for q, i in last_touch.items() if q <= 4), default=0)
    # cl columns the first two pairs need (generated before the loop); the
    # rest in two chunks emitted after tiles 3 and 11
    def _cols_before(i_lim):
        return max([k + 1 for k, (i, q) in enumerate(cols) if i < i_lim] or [1])
    ksplit = _cols_before(4)
    kmid = max(_cols_before(min(12, NT)), ksplit)
    cl_chunks = [(3, ksplit, kmid), (min(11, NT - 1), kmid, NCOL)]

    # per-core slo columns (-1 masks a token out of that column's region)
    # and per-core 1/count tail constants
    ident = np.eye(P, dtype=np.float32)
    s1 = np.eye(P, k=-1, dtype=np.float32)
    s2 = np.zeros((P, P), np.float32)
    s2[0, P - 1] = 1.0
    iota = np.broadcast_to(np.arange(P, dtype=np.float32), (P, P)).copy()
    ct = np.ascontiguousarray(np.concatenate([ident, s1, s2], axis=1))

    in_maps = []
    for b in range(B):
        slo_mat = np.full((P, NCOL), -1.0, np.float32)
        for k, (i, q) in enumerate(cols):
            u, c = REGIONS[q]
            sel = valid[b, i] & (u_of[b, i] == u) & (c_of[b, i] == c)
            slo_mat[sel, k] = (segt[b, i][sel] % P).astype(np.float32)
        cnt = np.bincount(seg[b][seg[b] <= 1024], minlength=1152).astype(np.float32)
        recm = np.zeros((P, NREG), np.float32)
        for q, (u, c) in enumerate(REGIONS):
            s_ids = u * P + np.arange(P)
            s_c = (s_ids > 512).astype(np.int64)
            ok = (cnt[s_ids] > 0) & (s_c == c) & (s_ids <= 1024)
            recm[ok, q] = 1.0 / cnt[s_ids][ok]
        biascol = np.full((P, 1), bias, np.float32)
        cc = np.concatenate([iota, slo_mat, recm, biascol], axis=1)
        in_maps.append({
            "x": np.ascontiguousarray(x[b, :Ltok]),
            "consts": np.ascontiguousarray(cc),
            "consts_tail": ct,
            "wrow": np.ascontiguousarray(cw.reshape(1, 2 * H)),
        })
    plan = {
        "ncol": NCOL,
        "ksplit": ksplit,
        "cl_chunks": cl_chunks,
        "passes": passes,
        "mms": [[tuple(m) for m in tm] for tm in mms],
        "emit_after": emit_after,
        "src_done_tile": src_done_tile,
    }
    return NT, plan, in_maps


def _run(inputs, trace=False, tmpdir=None):
    NT, plan, in_maps = _host_prep(inputs)
    nc = _build_nc(NT, plan)
    res = run_bass_kernel_spmd(nc, in_maps, core_ids=list(range(8)), trace=trace, tmpdir=tmpdir)
    out = np.stack([np.asarray(r["y"], dtype=np.float32) for r in res.results])
    return out, res


def kernel(**inputs) -> np.ndarray:
    out, _ = _run(inputs, trace=False)
    return out


if __name__ == "__main__":
    # CoreSim smoke test on core 0's inputs
    import jax
    jax.config.update("jax_platforms", "cpu")
    sys.path.insert(0, "/root/problem")
    import reference as ref
    from concourse.bass_interp import CoreSim

    inputs = ref.setup_inputs()
    NT, plan, in_maps = _host_prep(inputs)
    print("NT =", NT, "NCOL =", plan["ncol"], "ksplit =", plan["ksplit"],
          "src_done_tile =", plan["src_done_tile"])
    npass = sum(len(p) for p in plan["passes"])
    nmm = sum(len(m) for m in plan["mms"])
    print("proj passes:", npass, "matmuls:", nmm)
    nc = _build_nc(NT, plan)
    sim = CoreSim(nc)
    for name, arr in in_maps[0].items():
        sim.tensor(name)[:] = arr
    sim.simulate()
    got = np.array(sim.tensor("y"))
    expected = np.asarray(ref.reference(**inputs))[0]
    err = np.abs(got - expected).max()
    scale = np.abs(expected).max()
    print("CoreSim abs err:", err, "rel:", err / scale)
    assert err / scale < 1e-2, "CoreSim mismatch"
    print("CORESIM PASSES")


# revision 37
# speedup vs baseline: 1.0648x; 1.0648x over previous
"""Trainium2 Bass kernel for nn_BinaryTokenClassificationModel (segment_reduce).

Math: logits[b,i,j] = dot(segmean(1+i), w_src) + dot(segmean(513+j), w_tgt) + b,
where segmean(s) is the mean of outputs[b] over the s-th consecutive run of
equal word_ids.  dot commutes with the segment mean, so per-token projections
proj[t] = x[t]·w_c suffice.  Design notes (from HW traces):

- HBM-DMA-bound: only tokens with segment id <= 1024 matter (~10.5MB/core);
  the 16 per-core DMA engines cap at ~360 GB/s => ~29us floor.  The joint
  SBUF bandwidth (DMA write + DVE reads/writes + ACT read, ~20B per element)
  is the second roofline, so exactly ONE wide multiply stream runs: DVE
  multiplies (1.22us/tile), the scalar engine's fused activation-accumulate
  reduces (1.07us), per 1.43us DMA tile slot.  Pool's f32 tensor_tensor is
  ~4x slow and poisons DVE when co-run; tensor_tensor_reduce and Pool's
  scalar_tensor_tensor compile but die on real hardware; DVE->PSUM writes
  are slower than SBUF writes.
- consts ride at the head of the x DMA queue (FIFO per queue) so tile-0
  compute is never starved; tail-only selector consts are slotted behind the
  fifth x pair.
- Ragged segment-sums accumulate on the PE into a pre-zeroed PSUM tile with
  accumulate-only (start=False) [128,1]-rhs matmuls, emitted inline per tile
  (start=True flags would corrupt other open accumulations; with none, tiny
  matmuls interleave freely).  lhsT = on-chip-generated s_lo one-hots; the
  second (big) one-hot batch is generated after pair 1 so it never delays the
  first multiply.  1/count folds into host constants (word_ids metadata).
- The tail streams: each region's mean (one tensor_scalar from PSUM) and its
  tgt broadcast-staircase matmul emit as soon as the region's last tile is
  in; only the last chunk's chain + 4 broadcast-adds + stores trail the loop.

Sharding: pure data parallel, one example (B=8) per NeuronCore (8 cores).
"""
import sys

for _p in ("/opt/trn_rl_repo", "/root/.axon_site/_ro/trn_rl_repo"):
    if _p not in sys.path:
        sys.path.append(_p)

from contextlib import ExitStack

import numpy as np

import concourse.bacc as bacc
import concourse.bass as bass
import concourse.tile as tile
from concourse import mybir
from concourse.bass_utils import run_bass_kernel_spmd

F32 = mybir.dt.float32
BF16 = mybir.dt.bfloat16
P = 128
H = 1024
AL = mybir.AluOpType

# pool column regions: (seg_chunk u, c) with c: 0=src (segs 1..512), 1=tgt
# (segs 513..1024).  seg s -> chunk u = s//128, slo = s%128.
REGIONS = [(0, 0), (1, 0), (2, 0), (3, 0), (4, 0),
           (4, 1), (5, 1), (6, 1), (7, 1), (8, 1)]
NREG = len(REGIONS)


def _build_nc(NT: int, plan: dict) -> bass.Bass:
    NCOL = plan["ncol"]
    KSPLIT = plan["ksplit"]          # cl columns needed by the first two pairs
    CL_CHUNKS = plan["cl_chunks"]    # [(emit_after_tile, k0, k1)]
    passes = plan["passes"]          # per half-tile: list of c values
    mms = plan["mms"]                # per half-tile: list of (q, k, c, stop)
    emit_after = plan["emit_after"]  # per half-tile: regions closing at i
    src_done_tile = plan["src_done_tile"]

    nc = bacc.Bacc("TRN2", target_bir_lowering=False, debug=False, num_devices=8)
    NCE = NREG + 1                   # early f32 consts: rec | bias
    NCB = P + NCOL                   # early bf16 consts: iota | slo
    x_d = nc.declare_dram_parameter("x", [NT * P, H], F32, isOutput=False)
    ce_d = nc.declare_dram_parameter("consts", [P, NCE], F32, isOutput=False)
    cb_d = nc.declare_dram_parameter("consts16", [P, NCB], BF16, isOutput=False)
    ct_d = nc.declare_dram_parameter("consts_tail", [P, 3 * P], F32, isOutput=False)
    w_d = nc.declare_dram_parameter("wrow", [1, 2 * H], F32, isOutput=False)
    y_d = nc.declare_dram_parameter("y", [512, 512], F32, isOutput=True)

    with tile.TileContext(nc) as tc, ExitStack() as ctx:
        consts = ctx.enter_context(tc.tile_pool(name="consts", bufs=1))
        clp = ctx.enter_context(tc.tile_pool(name="clp", bufs=1))
        xpool = ctx.enter_context(tc.tile_pool(name="xp", bufs=9))
        scrv = ctx.enter_context(tc.tile_pool(name="scrv", bufs=3))
        vpool = ctx.enter_context(tc.tile_pool(name="vp", bufs=4))
        segp = ctx.enter_context(tc.tile_pool(name="segp", bufs=1))
        opool = ctx.enter_context(tc.tile_pool(name="op", bufs=4))
        psum_scr = ctx.enter_context(tc.tile_pool(name="pscr", bufs=2, space="PSUM"))
        ppool_acc = ctx.enter_context(tc.tile_pool(name="pacc", bufs=1, space="PSUM"))
        ppool_sm = ctx.enter_context(tc.tile_pool(name="psm", bufs=2, space="PSUM"))

        # ---- head of the sync DMA stream: w row (gates the PE broadcast),
        # early consts, then x pairs; FIFO per queue means nothing starves.
        wrow = consts.tile([1, 2 * H], F32)
        nc.sync.dma_start(out=wrow, in_=w_d[:])
        cc = consts.tile([P, NCE], F32)
        nc.sync.dma_start(out=cc, in_=ce_d[:])
        cb = consts.tile([P, NCB], BF16)
        nc.sync.dma_start(out=cb, in_=cb_d[:])

        iota = cb[:, 0:P]
        slo_mat = cb[:, P:P + NCOL]
        rec = cc[:, 0:NREG]
        biascol = cc[:, NCE - 1:NCE]
        ct = consts.tile([P, 3 * P], F32)
        ident = ct[:, 0:P]
        s1 = ct[:, P:2 * P]
        s2 = ct[:, 2 * P:3 * P]

        # ---- broadcast w across partitions: ones-row stationary matmuls into
        # PSUM, then scalar-engine copies into SBUF (idle engines at start).
        ones_row = consts.tile([1, P], F32)
        nc.gpsimd.memset(ones_row, 1.0)
        wrep = consts.tile([P, 2 * H], F32)
        for q in range(2):
            pw = psum_scr.tile([P, H], F32, tag="ps", name=f"pw{q}")
            for hh in range(2):
                lo = 1024 * q + 512 * hh
                nc.tensor.matmul(pw[:, 512 * hh:512 * (hh + 1)], lhsT=ones_row,
                                 rhs=wrow[:, lo:lo + 512], start=True, stop=True)
                nc.scalar.copy(out=wrep[:, lo:lo + 512],
                               in_=pw[:, 512 * hh:512 * (hh + 1)])

        # ---- s_lo one-hots (is_equal is DVE-only; bf16 in/out hits DVE's 2x
        # rate and halves the PE stationary loads): only the first two pairs'
        # columns up front; the rest follows after pairs 1 and 5.
        cl_all = clp.tile([P, NCOL, P], BF16)
        nc.vector.tensor_tensor(
            out=cl_all[:, 0:KSPLIT],
            in0=iota.unsqueeze(1).to_broadcast((P, KSPLIT, P)),
            in1=slo_mat[:, 0:KSPLIT].unsqueeze(2).to_broadcast((P, KSPLIT, P)),
            op=AL.is_equal)

        pool_ps = ppool_acc.tile([P, NREG], F32)
        nc.vector.memset(pool_ps, 0.0)

        mean = segp.tile([P, NREG], F32)
        msrc_ps = ppool_sm.tile([P, 4], F32, tag="sm")
        msrc = segp.tile([P, 4], F32)
        rowb_ps = ppool_sm.tile([P, 512], F32, tag="sm")
        # tgt region q -> rowb staircase block (col range, ident slice)
        rowb_blk = {5: (0, 127, (1, 128)), 6: (127, 255, (0, 128)),
                    7: (255, 383, (0, 128)), 8: (383, 511, (0, 128)),
                    9: (511, 512, (0, 1))}

        # ---- main loop ----
        for g in range(NT // 2):
            x_pair = xpool.tile([P, 2, H], F32, name="x_pair", tag="x_pair")
            src = x_d[256 * g:256 * (g + 1), :].rearrange("(two p) h -> p two h", p=P)
            nc.sync.dma_start(out=x_pair, in_=src)
            if g == min(4, NT // 2 - 1):
                # tail-only selector consts: behind five pairs of x (keeps the
                # head tight), well before the first staircase needs them
                nc.sync.dma_start(out=ct, in_=ct_d[:])
            for half in range(2):
                i = 2 * g + half
                x_sub = x_pair[:, half, :]
                vs = {}
                for c in passes[i]:
                    v = vpool.tile([P, 1], F32, name="v", tag="v")
                    scr = scrv.tile([P, H], F32, name="scrv")
                    nc.vector.tensor_tensor(
                        out=scr, in0=x_sub, in1=wrep[:, c * H:(c + 1) * H],
                        op=AL.mult)
                    # the reduce's mandatory wide `out` goes to PSUM so the
                    # only SBUF traffic is the scr read (SBUF bw is the
                    # binding resource); Pool casts v for the bf16 matmuls
                    dump = psum_scr.tile([P, H], F32, tag="ps", name="dump")
                    nc.scalar.activation(
                        out=dump, in_=scr,
                        func=mybir.ActivationFunctionType.Copy,
                        accum_out=v)
                    vb = vpool.tile([P, 1], BF16, name="vb", tag="vb")
                    nc.gpsimd.tensor_copy(out=vb, in_=v)
                    vs[c] = vb
                for (q, k, c, stop) in mms[i]:
                    nc.tensor.matmul(pool_ps[:, q:q + 1], lhsT=cl_all[:, k, :],
                                     rhs=vs[c], start=False, stop=stop,
                                     skip_group_check=True)
                for q in emit_after[i]:
                    if q >= 5:
                        # tgt mean on the scalar engine (AP scale), then its
                        # broadcast-staircase block
                        nc.scalar.activation(out=mean[:, q:q + 1],
                                             in_=pool_ps[:, q:q + 1],
                                             func=mybir.ActivationFunctionType.Copy,
                                             scale=rec[:, q:q + 1])
                        lo, hi, (a0, a1) = rowb_blk[q]
                        nc.tensor.matmul(rowb_ps[:, lo:hi],
                                         lhsT=mean[:, q:q + 1].to_broadcast((P, P)),
                                         rhs=ident[:, a0:a1], start=True, stop=True,
                                         skip_group_check=True)
                if i == src_done_tile:
                    # segs 1..512: batched means + s1/s2 selector matmuls +
                    # bias, all mid-loop
                    nc.vector.tensor_copy(out=mean[:, 0:5], in_=pool_ps[:, 0:5])
                    nc.vector.tensor_tensor(out=mean[:, 0:5], in0=mean[:, 0:5],
                                            in1=rec[:, 0:5], op=AL.mult)
                    nc.tensor.matmul(msrc_ps, lhsT=s1, rhs=mean[:, 0:4],
                                     start=True, stop=False, skip_group_check=True)
                    nc.tensor.matmul(msrc_ps, lhsT=s2, rhs=mean[:, 1:5],
                                     start=False, stop=True, skip_group_check=True)
                    nc.scalar.activation(out=msrc, in_=msrc_ps,
                                         func=mybir.ActivationFunctionType.Identity,
                                         bias=biascol, scale=1.0)
                # remaining one-hot chunks, emitted off the critical path in
                # pieces small enough not to lag the multiply stream
                for (at_i, k0, k1) in CL_CHUNKS:
                    if i == at_i and k1 > k0:
                        nc.vector.tensor_tensor(
                            out=cl_all[:, k0:k1],
                            in0=iota.unsqueeze(1).to_broadcast((P, k1 - k0, P)),
                            in1=slo_mat[:, k0:k1].unsqueeze(2).to_broadcast(
                                (P, k1 - k0, P)),
                            op=AL.is_equal)

        # ---- tail: 4 broadcast-adds + stores ----
        for k in range(4):
            lg = opool.tile([P, 512], F32, name="lg", tag="lg")
            if k % 2 == 0:
                nc.scalar.activation(out=lg, in_=rowb_ps,
                                     func=mybir.ActivationFunctionType.Identity,
                                     bias=msrc[:, k:k + 1], scale=1.0)
            else:
                nc.vector.tensor_scalar(out=lg, in0=rowb_ps, scalar1=msrc[:, k:k + 1],
                                        scalar2=None, op0=AL.add)
            nc.sync.dma_start(out=y_d[P * k:P * (k + 1), :], in_=lg)

    nc.compile()
    return nc


def _host_prep(inputs):
    x = np.ascontiguousarray(np.asarray(inputs["outputs"], dtype=np.float32))
    wid = np.asarray(inputs["word_ids"]).astype(np.int64)
    cw = np.asarray(inputs["classifier_w"], dtype=np.float32)
    bias = np.float32(np.asarray(inputs["classifier_b"]))
    B, L, Hd = x.shape
    assert (Hd, L) == (H, 4096) and B == 8
    assert int(inputs["num_src"]) == 512 and int(inputs["num_tgt"]) == 512

    # consecutive-run segment ids (attention_mask is all ones for this problem)
    new_seg = np.ones((B, L), np.int64)
    new_seg[:, 1:] = wid[:, 1:] != wid[:, :-1]
    seg = np.cumsum(new_seg, axis=1) - 1

    # token cutoff: segments beyond 1024 never reach the output
    cutoff = max(int(np.nonzero(seg[b] <= 1024)[0][-1]) for b in range(B))
    NPAIR = min((cutoff + 1 + 255) // 256, L // 256)
    NT = 2 * NPAIR
    Ltok = NT * P

    # per half-tile (128 contiguous tokens) region/column plan, union over cores
    segt = seg[:, :Ltok].reshape(B, NT, P)        # [B, NT, 128]
    valid = segt <= 1024
    u_of = segt // P
    c_of = (segt > 512).astype(np.int64)          # 0=src, 1=tgt
    qidx = {r: q for q, r in enumerate(REGIONS)}

    cols = []                        # (i, q) -> column index k
    mms = []                         # per i: [(q, k, c, stop)] (stop set below)
    passes = []                      # per i: list of c values
    last_touch = {}                  # q -> last tile index
    for i in range(NT):
        regs = set()
        for b in range(B):
            vb = valid[b, i]
            if not vb.any():
                continue
            for u, c in zip(u_of[b, i][vb], c_of[b, i][vb]):
                regs.add((int(u), int(c)))
        regs = sorted(regs, key=lambda r: qidx[r])
        tile_mms = []
        for r in regs:
            q = qidx[r]
            k = len(cols)
            cols.append((i, q))
            tile_mms.append([q, k, r[1], False])
            last_touch[q] = i
        mms.append(tile_mms)
        passes.append(sorted({r[1] for r in regs}))

    emit_after = [[] for _ in range(NT)]
    for q, i in last_touch.items():
        emit_after[i].append(q)
        for m in mms[i]:
            if m[0] == q:
                m[3] = True
    for e in emit_after:
        e.sort()
    NCOL = len(cols)
    src_done_tile = max((i for q, i in last_touch.items() if q <= 4), default=0)
    # cl columns the first two pairs need (generated before the loop); the
    # rest in two chunks emitted after tiles 3 and 11
    def _cols_before(i_lim):
        return max([k + 1 for k, (i, q) in enumerate(cols) if i < i_lim] or [1])
    ksplit = _cols_before(4)
    kmid = max(_cols_before(min(12, NT)), ksplit)
    cl_chunks = [(3, ksplit, kmid), (min(11, NT - 1), kmid, NCOL)]

    # per-core slo columns (-1 masks a token out of that column's region)
    # and per-core 1/count tail constants
    ident = np.eye(P, dtype=np.float32)
    s1 = np.eye(P, k=-1, dtype=np.float32)
    s2 = np.zeros((P, P), np.float32)
    s2[0, P - 1] = 1.0
    iota = np.broadcast_to(np.arange(P, dtype=np.float32), (P, P)).copy()
    ct = np.ascontiguousarray(np.concatenate([ident, s1, s2], axis=1))

    try:
        import ml_dtypes
        bf16 = ml_dtypes.bfloat16
    except ImportError:  # pragma: no cover
        import jax.numpy as jnp
        bf16 = jnp.bfloat16
    in_maps = []
    for b in range(B):
        slo_mat = np.full((P, NCOL), -1.0, np.float32)
        for k, (i, q) in enumerate(cols):
            u, c = REGIONS[q]
            sel = valid[b, i] & (u_of[b, i] == u) & (c_of[b, i] == c)
            slo_mat[sel, k] = (segt[b, i][sel] % P).astype(np.float32)
        cnt = np.bincount(seg[b][seg[b] <= 1024], minlength=1152).astype(np.float32)
        recm = np.zeros((P, NREG), np.float32)
        for q, (u, c) in enumerate(REGIONS):
            s_ids = u * P + np.arange(P)
            s_c = (s_ids > 512).astype(np.int64)
            ok = (cnt[s_ids] > 0) & (s_c == c) & (s_ids <= 1024)
            recm[ok, q] = 1.0 / cnt[s_ids][ok]
        biascol = np.full((P, 1), bias, np.float32)
        cc = np.concatenate([recm, biascol], axis=1)
        cb = np.concatenate([iota, slo_mat], axis=1).astype(bf16)
        in_maps.append({
            "x": np.ascontiguousarray(x[b, :Ltok]),
            "consts": np.ascontiguousarray(cc),
            "consts16": np.ascontiguousarray(cb),
            "consts_tail": ct,
            "wrow": np.ascontiguousarray(cw.reshape(1, 2 * H)),
        })
    plan = {
        "ncol": NCOL,
        "ksplit": ksplit,
        "cl_chunks": cl_chunks,
        "passes": passes,
        "mms": [[tuple(m) for m in tm] for tm in mms],
        "emit_after": emit_after,
        "src_done_tile": src_done_tile,
    }
    return NT, plan, in_maps


def _run(inputs, trace=False, tmpdir=None):
    NT, plan, in_maps = _host_prep(inputs)
    nc = _build_nc(NT, plan)
    res = run_bass_kernel_spmd(nc, in_maps, core_ids=list(range(8)), trace=trace, tmpdir=tmpdir)
    out = np.stack([np.asarray(r["y"], dtype=np.float32) for r in res.results])
    return out, res


def kernel(**inputs) -> np.ndarray:
    out, _ = _run(inputs, trace=False)
    return out


if __name__ == "__main__":
    # CoreSim smoke test on core 0's inputs
    import jax
    jax.config.update("jax_platforms", "cpu")
    sys.path.insert(0, "/root/problem")
    import reference as ref
    from concourse.bass_interp import CoreSim

    inputs = ref.setup_inputs()
    NT, plan, in_maps = _host_prep(inputs)
    print("NT =", NT, "NCOL =", plan["ncol"], "ksplit =", plan["ksplit"],
          "src_done_tile =", plan["src_done_tile"])
    npass = sum(len(p) for p in plan["passes"])
    nmm = sum(len(m) for m in plan["mms"])
    print("proj passes:", npass, "matmuls:", nmm)
    nc = _build_nc(NT, plan)
    sim = CoreSim(nc)
    for name, arr in in_maps[0].items():
        sim.tensor(name)[:] = arr
    sim.simulate()
    got = np.array(sim.tensor("y"))
    expected = np.asarray(ref.reference(**inputs))[0]
    err = np.abs(got - expected).max()
    scale = np.abs(expected).max()
    print("CoreSim abs err:", err, "rel:", err / scale)
    assert err / scale < 1e-2, "CoreSim mismatch"
    print("CORESIM PASSES")
